# revision 37
# baseline (speedup 1.0000x reference)
"""CAM (channel attention module) Bass kernel for Trainium2.

Problem: y = gamma * (softmax_rev(v @ v.T * s) @ v) + x per batch sample,
with x [16, 128, 128, 128] f32, v = x.reshape(B, C, H*W).

Sharding: pure data parallel — B=16 split as 2 samples per core across
8 NeuronCores; gamma replicated; no collectives.

HBM traffic strategy: the module is memory-bound, so both x and y cross
HBM as bf16 (host casts f32<->bf16 around the device call).  Compute was
already all-bf16 on the PE in the f32-I/O version, so the only added
error is bf16 rounding of the residual x and of the stored y — ~0.2% of
per-element magnitude each, far inside the 2e-2 gate.  This halves the
HBM bytes per core (33.6 MB -> 16.8 MB) and deletes the entire bf16
rhs-copy production stage of the f32 version (x arrives in SBUF already
bf16, so gram transposes, attention rhs, and the residual add all read
the loaded x tiles directly).

Per-core dataflow (per sample, [C=128, HW=16384] bf16):
  1. DMA both samples into SBUF up front (quarter-loads) so the input
     stream never stalls behind output DMAs.
  2. Gram matrix E = V V^T: PE transposes bf16 chunks (4 per PSUM bank),
     one ACT/DVE copy PSUM->SBUF per group, PE accumulates vT.T @ vT into
     a PSUM bank (bf16 inputs, f32 accumulate).
  3. Reversed softmax: rowmin of E (DVE), p = exp(-s*E + s*rowmin) with
     fused row-sum Z (single ACT op), r = 1/Z (DVE), fold gamma: S' =
     p * (gamma*r) per row; PE-transpose -> bf16 stationary S'T.
  4. Attention: psum = S'T.T @ x_chunk (32 matmuls, N=512); y chunk =
     psum + x chunk (DVE add, bf16 out) -> batched 1MB bf16 DMA out.
  Sample 0's attention phase is interleaved with sample 1's Gram phase in
  emission order so the PE/ACT streams of the two samples overlap.
"""

import os as _os
import shutil as _shutil
import tempfile as _tempfile

import numpy as np

# The libneuronxla NEFF cache key does not cover the Bass BIR embedded in
# the jit custom call: two different Bass programs with the same outer HLO
# (same shapes/dtypes) collide, silently serving the wrong NEFF. Point the
# cache at a private fresh dir before the first compile in this process,
# and drop any pre-existing default caches.
if not _os.environ.get("CAM_NEFF_CACHE_SET"):
    _os.environ["NEURON_COMPILE_CACHE_URL"] = _tempfile.mkdtemp(
        prefix="cam_neffcache_")
    _os.environ["CAM_NEFF_CACHE_SET"] = "1"
    for _p in ("/var/tmp/neuron-compile-cache",
               _os.path.expanduser("~/.neuron-compile-cache")):
        _shutil.rmtree(_p, ignore_errors=True)

B, C, H, W = 16, 128, 128, 128
HW = H * W
N_CORES = 8
B_PER = B // N_CORES  # 2 samples per core
SCALE = 1.0 / float(np.sqrt(np.float32(HW)))  # 1/128

# tuning knobs (exp scripts override these before building variants)
CFG = {
    "store_engine": "sync",  # "sync" | "scalar" | "gpsimd"
    "att_copy": ("dve", "act"),  # per-block PSUM->SBUF copy engine cycle
    "ps_t": 2,  # [128,2048] tiles: 2 PSUM banks each
    "ps_a": 3,
    "xq_bufs": 12,
    "outp_bufs": 3,
    "gram_lead": 4,
    "vt_bufs": 5,  # [128,2048] tiles
}

NQ = 4  # x quarter-loads per sample
QF = HW // NQ  # 4096 bf16 per quarter
ATT_N = 512  # attention matmul moving free dim (one PSUM bank)
OUT_BLK = 2048  # attention/output block width
SG = 1024  # gram super-group width (8 transposed chunks per PSUM bank)
N_SG = HW // SG  # 16 gram super-groups per sample
GRAM_LEAD = 2  # super-groups of transpose lead over the gram matmuls (default)
N_BLOCKS = HW // OUT_BLK  # 8 attention/output blocks
WARM = 8  # PE warm-up matmuls (cold-start clock ramp only)


class _SampleCtx:
    """Per-sample tiles threaded between the emission phases."""

    def __init__(self):
        self.xq = None
        self.eps = None
        self.sprime = None
        self.spT = None
        self.ot = None  # current [128, 2*OUT_BLK] output tile
        self.pt2 = None  # current [128, 2*SG] transpose PSUM pair tile
        self.vt = [None] * N_SG  # transposed bf16 super-group tiles


def _emit_load(nc, mybir, pools, x_d, b, sc, split_first=False):
    bf16 = mybir.dt.bfloat16
    xpool = pools["xpool"]
    sc.xq = []
    for q in range(NQ):
        xt = xpool.tile([128, QF], bf16, tag="xq")
        if pools.get("_mode") not in (None, "full", "dma"):
            # token write so the tile framework sees the tile as produced;
            # compute then reads (mostly garbage) SBUF with no DMA cost
            nc.sync.dma_start(out=xt[:, :16], in_=x_d[b, :, :16])
            sc.xq.append(xt)
            continue
        if q == 0 and split_first:
            # halve the first transfer so the gram phase starts earlier
            h = QF // 2
            nc.sync.dma_start(out=xt[:, :h], in_=x_d[b, :, :h])
            nc.sync.dma_start(out=xt[:, h:], in_=x_d[b, :, h:QF])
        else:
            nc.sync.dma_start(out=xt, in_=x_d[b, :, q * QF : (q + 1) * QF])
        sc.xq.append(xt)


def _emit_gram_T(nc, mybir, pools, sc, sg, engine="dve"):
    """Transpose half of one gram super-group: 8 PE transposes of [128,128]
    bf16 x chunks into half of a [128,2048] 2-bank PSUM tile.  On the odd
    sg of each pair, ONE [128,2048] PSUM->SBUF bf16 copy drains both banks
    (wider copies amortize the per-instruction fixed cost; DVE runs
    all-bf16 copies at 2x, ACT at 1x)."""
    bf16 = mybir.dt.bfloat16
    src = sc.xq[sg // (QF // SG)]
    scol = (sg % (QF // SG)) * SG
    if sg % 2 == 0:
        sc.pt2 = pools["ps_t"].tile([128, 2 * SG], bf16, tag="pt")
    base = (sg % 2) * SG
    pt2 = sc.pt2
    for i in range(8):
        nc.tensor.matmul(
            pt2[:, base + i * 128 : base + (i + 1) * 128],
            src[:, scol + i * 128 : scol + (i + 1) * 128],
            pools["ident_bf16"],
            is_transpose=True,
            skip_group_check=True,
        )
    if sg % 2 == 1:
        vt = pools["vt"].tile([128, 2 * SG], bf16)
        if engine == "act":
            nc.scalar.copy(vt, pt2)
        else:
            nc.vector.tensor_copy(vt, pt2)
        sc.vt[sg - 1] = (vt, 0)
        sc.vt[sg] = (vt, SG)


def _emit_gram_MM(nc, mybir, pools, sc, sg):
    """Matmul half of one super-group: 8 accumulating vT.T @ vT matmuls."""
    f32 = mybir.dt.float32
    if sc.eps is None:
        sc.eps = pools["ps_g"].tile([128, 128], f32)
    vt, off = sc.vt[sg]
    for i in range(8):
        k = sg * 8 + i
        vti = vt[:, off + i * 128 : off + (i + 1) * 128]
        nc.tensor.matmul(
            sc.eps, vti, vti, start=(k == 0), stop=(k == 8 * N_SG - 1),
            skip_group_check=True,
        )
    sc.vt[sg] = None


def _emit_softmax_chain(nc, mybir, pools, sc):
    """Reversed softmax + gamma fold, ACT/DVE part (through sprime).

    Produces sprime = S' + I where S' = gamma * softmax_rev: with the
    identity folded in, the attention matmul computes gamma*(A @ V) + V
    directly, so the per-element residual add disappears (the
    V-passthrough is exact: 1.0 * bf16 V accumulated in f32 PSUM)."""
    f32 = mybir.dt.float32
    sm_pool = pools["sm"]
    eps = sc.eps
    rowmin = sm_pool.tile([128, 1], f32)
    nc.vector.tensor_reduce(
        rowmin, eps, axis=mybir.AxisListType.X, op=mybir.AluOpType.min
    )
    biasv = sm_pool.tile([128, 1], f32)
    nc.scalar.mul(biasv, rowmin, SCALE)
    p_sb = sm_pool.tile([128, 128], f32)
    zsum = sm_pool.tile([128, 1], f32)
    nc.scalar.activation(
        p_sb, eps, mybir.ActivationFunctionType.Exp,
        bias=biasv, scale=-SCALE, accum_out=zsum,
    )
    rz = sm_pool.tile([128, 1], f32)
    nc.vector.reciprocal(rz, zsum)
    rzg = sm_pool.tile([128, 1], f32)
    nc.vector.tensor_mul(rzg, rz, pools["gamma_sb"])
    # S' + I = (p * (gamma/Z)) + I in one fused DVE op
    sprime = sm_pool.tile([128, 128], f32)
    nc.vector.scalar_tensor_tensor(
        sprime, in0=p_sb, scalar=rzg, in1=pools["ident_f32"],
        op0=mybir.AluOpType.mult, op1=mybir.AluOpType.add,
    )
    sc.sprime = sprime


def _emit_softmax_T(nc, mybir, pools, sc):
    """PE transpose of sprime -> bf16 stationary (S' + I)^T."""
    f32 = mybir.dt.float32
    bf16 = mybir.dt.bfloat16
    pst = pools["ps_t"].tile([128, 512], f32, tag="pt")
    nc.tensor.matmul(pst[:, 0:128], sc.sprime, pools["ident_f32"],
                     is_transpose=True, skip_group_check=True)
    spT = pools["sm"].tile([128, 128], bf16)
    nc.vector.tensor_copy(spT, pst[:, 0:128])
    sc.spT = spT


def _emit_softmax(nc, mybir, pools, sc):
    _emit_softmax_chain(nc, mybir, pools, sc)
    _emit_softmax_T(nc, mybir, pools, sc)


def _emit_attn_block(nc, mybir, pools, y_d, b, sc, j, copy_engines=("act",),
                     store_engine="sync", ot_tile=None, do_store=True):
    """One [128, OUT_BLK] attention(+folded residual) block.  PSUM already
    holds the final y values ((S'+I) @ V); each [128, 2*ATT_N] PSUM tile is
    cast PSUM f32 -> SBUF bf16 by one ACT/DVE copy.  Output tiles span two
    blocks ([128, 2*OUT_BLK] bf16) so stores are 1MB DMAs; the DMA issues
    after the odd block of each pair.  store_engine="scalar" puts the store
    on the ACT HWDGE ring so it can't head-of-line-block loads on the SP
    ring."""
    bf16 = mybir.dt.bfloat16
    if j % 2 == 0:
        sc.ot = ot_tile if ot_tile is not None else pools["outp"].tile(
            [128, 2 * OUT_BLK], bf16, tag="ot")
    obase = (j % 2) * OUT_BLK
    xt = sc.xq[j // 2]
    xbase = (j % 2) * OUT_BLK
    for pp in range(OUT_BLK // ATT_N):  # N=512 chunks (one PSUM bank each)
        pa = pools["ps_a"].tile([128, ATT_N], mybir.dt.float32)
        off = xbase + pp * ATT_N
        nc.tensor.matmul(
            pa, sc.spT, xt[:, off : off + ATT_N], skip_group_check=True)
        osl = sc.ot[:, obase + pp * ATT_N : obase + (pp + 1) * ATT_N]
        if copy_engines[pp % len(copy_engines)] == "act":
            nc.scalar.copy(osl, pa)
        else:
            nc.vector.tensor_copy(osl, pa)
    if j % 2 == 1 and do_store and pools.get("_mode") != "nodma":
        eng = {"scalar": nc.scalar, "gpsimd": nc.gpsimd, "sync": nc.sync}[
            store_engine]
        eng.dma_start(
            out=y_d[b, :, (j - 1) * OUT_BLK : (j + 1) * OUT_BLK], in_=sc.ot)


def _emit_dma_only(nc, mybir, pools, x_d, y_d):
    """Loads + stores only (bandwidth floor measurement): store back the
    loaded x tiles (same transfer sizes/count as the real kernel)."""
    s0, s1 = _SampleCtx(), _SampleCtx()
    _emit_load(nc, mybir, pools, x_d, 0, s0, split_first=True)
    _emit_load(nc, mybir, pools, x_d, 1, s1)
    for b, sc in ((0, s0), (1, s1)):
        for q in range(NQ):
            nc.sync.dma_start(
                out=y_d[b, :, q * QF : (q + 1) * QF], in_=sc.xq[q])


def _emit_phased(nc, mybir, pools, x_d, y_d, mode):
    """Phase-isolation variants for attribution experiments.

    "t": transposes+copies only; "tg": + gram MMs + softmax;
    "attn": attention blocks only (fake stationary)."""
    bf16 = mybir.dt.bfloat16
    s0, s1 = _SampleCtx(), _SampleCtx()
    _emit_load(nc, mybir, pools, x_d, 0, s0, split_first=True)
    _emit_load(nc, mybir, pools, x_d, 1, s1)
    if mode in ("t", "tg"):
        for sc in (s0, s1):
            for sg in range(N_SG):
                _emit_gram_T(nc, mybir, pools, sc, sg,
                             engine="dve" if sg % 2 == 0 else "act")
                if mode == "tg" and sg >= GRAM_LEAD:
                    _emit_gram_MM(nc, mybir, pools, sc, sg - GRAM_LEAD)
            if mode == "tg":
                for sg in range(N_SG - GRAM_LEAD, N_SG):
                    _emit_gram_MM(nc, mybir, pools, sc, sg)
                _emit_softmax(nc, mybir, pools, sc)
    elif mode == "attn":
        for sc in (s0, s1):
            spT = pools["sm"].tile([128, 128], bf16)
            nc.scalar.copy(spT, pools["ident_bf16"])
            sc.spT = spT
            for j in range(N_BLOCKS):
                _emit_attn_block(nc, mybir, pools, y_d, 0, sc, j,
                                 copy_engines=("act", "dve"))
    elif mode == "tno":
        # pure PE transpose rate: 256 transposes, no PSUM->SBUF copies
        for sc in (s0, s1):
            for sg in range(N_SG):
                src = sc.xq[sg // (QF // SG)]
                scol = (sg % (QF // SG)) * SG
                pt = pools["ps_t"].tile([128, SG], bf16, tag="pt")
                for i in range(8):
                    nc.tensor.matmul(
                        pt[:, i * 128 : (i + 1) * 128],
                        src[:, scol + i * 128 : scol + (i + 1) * 128],
                        pools["ident_bf16"],
                        is_transpose=True,
                        skip_group_check=True,
                    )
    elif mode == "gno":
        # pure LDW+MM rate: 256 accumulating gram matmuls, stationary
        # alternates between two static tiles (forces a weight load each)
        va = pools["sm"].tile([128, 128], bf16)
        vb = pools["sm"].tile([128, 128], bf16)
        nc.scalar.copy(va, pools["ident_bf16"])
        nc.scalar.copy(vb, pools["ident_bf16"])
        f32 = mybir.dt.float32
        for s in range(B_PER):
            eps = pools["ps_g"].tile([128, 128], f32)
            for k in range(8 * N_SG):
                vt = va if k % 2 == 0 else vb
                nc.tensor.matmul(
                    eps, vt, vt, start=(k == 0), stop=(k == 8 * N_SG - 1),
                    skip_group_check=True,
                )
    elif mode == "ano":
        # pure attention MM rate: 64 N=512 MMs, fixed stationary, no copies
        spT = pools["sm"].tile([128, 128], bf16)
        nc.scalar.copy(spT, pools["ident_bf16"])
        for sc in (s0, s1):
            for j in range(N_BLOCKS):
                xt = sc.xq[j // 2]
                xbase = (j % 2) * OUT_BLK
                for pp in range(OUT_BLK // ATT_N):
                    pa = pools["ps_a"].tile([128, ATT_N], mybir.dt.float32)
                    off = xbase + pp * ATT_N
                    nc.tensor.matmul(
                        pa, spT, xt[:, off : off + ATT_N],
                        skip_group_check=True)
    else:
        raise ValueError(mode)


def _emit_workload(nc, mybir, pools, x_d, y_d, mode="full", carried=None):
    """Both samples, software-pipelined in emission order."""
    if mode == "dma":
        return _emit_dma_only(nc, mybir, pools, x_d, y_d)
    if mode not in ("full", "nodma"):
        return _emit_phased(nc, mybir, pools, x_d, y_d, mode)
    f32 = mybir.dt.float32
    s0, s1 = _SampleCtx(), _SampleCtx()

    # PE warm-up: a few dependency-free matmuls during the load head help
    # the cold-start clock ramp; kept short because in the steady-state
    # rep loop they are pure overhead (PE is already hot).
    warm = pools["ps_t"].tile([128, 128], f32, tag="pt")
    for w in range(WARM):
        nc.tensor.matmul(warm, pools["ident_bf16"], pools["ident_bf16"],
                         skip_group_check=True)

    if carried is None:
        _emit_load(nc, mybir, pools, x_d, 0, s0, split_first=True)
        _emit_load(nc, mybir, pools, x_d, 1, s1)
    else:
        # Deferred-store pipeline (rep-loop body): the previous body's
        # sample-1 output tiles are stored HERE, interleaved between this
        # body's loads on the same sync HWDGE ring.  This keeps the DMA
        # stream gapless across the loop back-edge: without it the next
        # body's loads queue behind ALL of this body's stores in ring
        # order, serializing the pipeline at ~(stores tail + loads head).
        bf16 = mybir.dt.bfloat16
        xpool = pools["xpool"]
        s0.xq, s1.xq = [], []
        for sc in (s0, s1):
            for q in range(NQ):
                xt = xpool.tile([128, QF], bf16, tag="xq")
                sc.xq.append(xt)

        def _ld(sc, b, q):
            nc.sync.dma_start(out=sc.xq[q], in_=x_d[b, :, q * QF:(q + 1) * QF])

        def _st(p):
            nc.sync.dma_start(
                out=y_d[1, :, p * 2 * OUT_BLK:(p + 1) * 2 * OUT_BLK],
                in_=carried[p])

        h = QF // 2
        nc.sync.dma_start(out=s0.xq[0][:, :h], in_=x_d[0, :, :h])
        nc.sync.dma_start(out=s0.xq[0][:, h:], in_=x_d[0, :, h:QF])
        _ld(s0, 0, 1)
        _st(0)
        _ld(s0, 0, 2)
        _st(1)
        _ld(s0, 0, 3)
        _st(2)
        _ld(s1, 1, 0)
        _st(3)
        _ld(s1, 1, 1)
        _ld(s1, 1, 2)
        _ld(s1, 1, 3)

    # gram copy engine pattern: 2/3 DVE (bf16 copies run 2x there), 1/3 ACT
    def _geng(sg):
        return "act" if sg % 3 == 2 else "dve"

    # sample-0 gram runs alone, software-pipelined with `lead` super-groups
    # of transpose lead so the PSUM->SBUF copy round-trip of super-group g
    # hides behind the transposes of g+1..g+lead.
    lead = CFG.get("gram_lead", GRAM_LEAD)
    for sg in range(N_SG):
        _emit_gram_T(nc, mybir, pools, s0, sg, engine=_geng(sg))
        if sg >= lead:
            _emit_gram_MM(nc, mybir, pools, s0, sg - lead)
    for sg in range(N_SG - lead, N_SG):
        _emit_gram_MM(nc, mybir, pools, s0, sg)

    # softmax-0 ACT/DVE chain runs while the PE starts sample-1 transposes
    _emit_softmax_chain(nc, mybir, pools, s0)
    _emit_gram_T(nc, mybir, pools, s1, 0, engine=_geng(0))
    _emit_gram_T(nc, mybir, pools, s1, 1, engine=_geng(1))
    _emit_softmax_T(nc, mybir, pools, s0)

    # interleave: sample-0 attention blocks (j=0..6) with the remaining
    # sample-1 gram super-groups.  A-copies alternate ACT/DVE.
    t_idx, mm_idx = 2, 0
    for j in range(N_BLOCKS - 1):
        _emit_attn_block(nc, mybir, pools, y_d, 0, s0, j,
                         copy_engines=CFG["att_copy"], store_engine=CFG["store_engine"])
        for _ in range(2):
            if t_idx < N_SG:
                _emit_gram_T(nc, mybir, pools, s1, t_idx, engine=_geng(t_idx))
                t_idx += 1
            if mm_idx < t_idx - lead and mm_idx < N_SG:
                _emit_gram_MM(nc, mybir, pools, s1, mm_idx)
                mm_idx += 1
    while mm_idx < N_SG:
        _emit_gram_MM(nc, mybir, pools, s1, mm_idx)
        mm_idx += 1

    # softmax-1 chain overlaps the last sample-0 attention block on the PE
    _emit_softmax_chain(nc, mybir, pools, s1)
    _emit_attn_block(nc, mybir, pools, y_d, 0, s0, N_BLOCKS - 1,
                     copy_engines=CFG["att_copy"], store_engine=CFG["store_engine"])
    _emit_softmax_T(nc, mybir, pools, s1)

    for j in range(N_BLOCKS):
        _emit_attn_block(
            nc, mybir, pools, y_d, 1, s1, j,
            copy_engines=CFG["att_copy"], store_engine=CFG["store_engine"],
            ot_tile=(carried[j // 2] if (carried is not None and j % 2 == 0)
                     else None),
            do_store=(carried is None))


def _build_bass(reps=0, unroll=1, mode="full", flat=1):
    """Build the Bass program. reps>0 wraps the workload in a HW loop that
    repeats it (for steady-state benchmarking; output is idempotent);
    unroll>1 amortizes the loop back-edge (barrier + IRAM refetch).
    mode: "full" | "dma" (loads+stores only) | "nodma" (compute only)."""
    import concourse.bacc as bacc
    import concourse.tile as tile
    from concourse import masks, mybir
    from contextlib import ExitStack

    f32 = mybir.dt.float32
    bf16 = mybir.dt.bfloat16

    # Bacc (not plain Bass): its compile() runs generate_event_semaphores,
    # which splits multi-wait instructions — walrus rejects them on TRN2.
    nc = bacc.Bacc(
        "TRN2",
        target_bir_lowering=False,
        debug=False,
        enable_asserts=False,
        num_devices=N_CORES,
    )
    x_d = nc.dram_tensor("x", [B_PER, C, HW], bf16, kind="ExternalInput")
    g_d = nc.dram_tensor("gamma", [1], f32, kind="ExternalInput")
    y_d = nc.dram_tensor("y", [B_PER, C, HW], bf16, kind="ExternalOutput")

    with tile.TileContext(nc) as tc, ExitStack() as ctx:
        pools = {}
        for name, kw in [
            ("consts", dict(bufs=1)),
            ("xpool", dict(bufs=CFG["xq_bufs"])),
            ("vt", dict(bufs=CFG.get("vt_bufs", 6))),
            ("sm", dict(bufs=4)),
            ("outp", dict(bufs=CFG["outp_bufs"])),
            ("ps_t", dict(bufs=CFG["ps_t"], space="PSUM")),  # [128,1024] bf16: 1 bank each
            ("ps_g", dict(bufs=1, space="PSUM")),
            ("ps_a", dict(bufs=CFG["ps_a"], space="PSUM")),  # [128,512] f32: 1 bank each
        ]:
            pools[name] = ctx.enter_context(tc.tile_pool(name=name, **kw))

        ident_f32 = pools["consts"].tile([128, 128], f32)
        masks.make_identity(nc, ident_f32)
        ident_bf16 = pools["consts"].tile([128, 128], mybir.dt.bfloat16)
        masks.make_identity(nc, ident_bf16)
        gamma_sb = pools["consts"].tile([128, 1], f32)
        nc.gpsimd.dma_start(out=gamma_sb, in_=g_d[:].to_broadcast((128, 1)))
        pools["ident_f32"] = ident_f32
        pools["ident_bf16"] = ident_bf16
        pools["gamma_sb"] = gamma_sb

        pools["_mode"] = mode
        if flat > 1:
            # loop-free repetition for TimelineSim period analysis
            carried = None
            if mode == "full":
                outd = ctx.enter_context(tc.tile_pool(name="outd", bufs=1))
                carried = []
                for p in range(4):
                    t = outd.tile([128, 2 * OUT_BLK], bf16, tag=f"od{p}")
                    nc.gpsimd.memset(t, 0)
                    carried.append(t)
            for _ in range(flat):
                _emit_workload(nc, mybir, pools, x_d, y_d, mode=mode,
                               carried=carried)
        elif reps:
            carried = None
            if mode == "full":
                # persistent sample-1 output tiles for the deferred-store
                # pipeline; zero-filled so iteration 0's deferred stores
                # read initialized data (overwritten by iterations >= 1)
                outd = ctx.enter_context(tc.tile_pool(name="outd", bufs=1))
                carried = []
                for p in range(4):
                    t = outd.tile([128, 2 * OUT_BLK], bf16, tag=f"od{p}")
                    nc.gpsimd.memset(t, 0)
                    carried.append(t)
            # PE body is several hundred instructions (> 1 IRAM block):
            # hint the back-edge prefetch so the bench loop doesn't pay an
            # I$ miss.
            with tc.For_i(0, reps, 1, hint_engines=(mybir.EngineType.PE,)):
                for _ in range(unroll):
                    _emit_workload(nc, mybir, pools, x_d, y_d, mode=mode,
                                   carried=carried)
        else:
            _emit_workload(nc, mybir, pools, x_d, y_d, mode=mode)

    nc.compile()
    return nc


_NC_CACHE = None


def _get_nc():
    global _NC_CACHE
    if _NC_CACHE is None:
        _NC_CACHE = _build_bass()
    return _NC_CACHE


def kernel(x, gamma, trace=False):
    from concourse.bass_utils import run_bass_kernel_spmd
    from concourse import mybir

    np_bf16 = mybir.dt.np(mybir.dt.bfloat16)
    x = np.asarray(x, dtype=np.float32).astype(np_bf16)
    gamma = np.asarray(gamma, dtype=np.float32)
    nc = _get_nc()

    xs = x.reshape(N_CORES, B_PER, C, HW)
    in_maps = [{"x": xs[i], "gamma": gamma} for i in range(N_CORES)]
    res = run_bass_kernel_spmd(nc, in_maps, core_ids=list(range(N_CORES)), trace=trace)
    out = np.stack([res.results[i]["y"] for i in range(N_CORES)], axis=0)
    out = out.astype(np.float32).reshape(B, C, H, W)
    if trace:
        return out, res
    return out



# revision 39
# speedup vs baseline: 1.0777x; 1.0777x over previous
"""CAM (channel attention module) Bass kernel for Trainium2.

Problem: y = gamma * (softmax_rev(v @ v.T * s) @ v) + x per batch sample,
with x [16, 128, 128, 128] f32, v = x.reshape(B, C, H*W).

Sharding: pure data parallel — B=16 split as 2 samples per core across
8 NeuronCores; gamma replicated; no collectives.

HBM traffic strategy: the module is memory-bound, so both x and y cross
HBM as bf16 (host casts f32<->bf16 around the device call).  Compute was
already all-bf16 on the PE in the f32-I/O version, so the only added
error is bf16 rounding of the residual x and of the stored y — ~0.2% of
per-element magnitude each, far inside the 2e-2 gate.  This halves the
HBM bytes per core (33.6 MB -> 16.8 MB) and deletes the entire bf16
rhs-copy production stage of the f32 version (x arrives in SBUF already
bf16, so gram transposes, attention rhs, and the residual add all read
the loaded x tiles directly).

Per-core dataflow (per sample, [C=128, HW=16384] bf16):
  1. DMA both samples into SBUF up front (quarter-loads) so the input
     stream never stalls behind output DMAs.
  2. Gram matrix E = V V^T: PE transposes bf16 chunks (4 per PSUM bank),
     one ACT/DVE copy PSUM->SBUF per group, PE accumulates vT.T @ vT into
     a PSUM bank (bf16 inputs, f32 accumulate).
  3. Reversed softmax: rowmin of E (DVE), p = exp(-s*E + s*rowmin) with
     fused row-sum Z (single ACT op), r = 1/Z (DVE), fold gamma: S' =
     p * (gamma*r) per row; PE-transpose -> bf16 stationary S'T.
  4. Attention: psum = S'T.T @ x_chunk (32 matmuls, N=512); y chunk =
     psum + x chunk (DVE add, bf16 out) -> batched 1MB bf16 DMA out.
  Sample 0's attention phase is interleaved with sample 1's Gram phase in
  emission order so the PE/ACT streams of the two samples overlap.
"""

import os as _os
import shutil as _shutil
import tempfile as _tempfile

import numpy as np

# The libneuronxla NEFF cache key does not cover the Bass BIR embedded in
# the jit custom call: two different Bass programs with the same outer HLO
# (same shapes/dtypes) collide, silently serving the wrong NEFF. Point the
# cache at a private fresh dir before the first compile in this process,
# and drop any pre-existing default caches.
if not _os.environ.get("CAM_NEFF_CACHE_SET"):
    _os.environ["NEURON_COMPILE_CACHE_URL"] = _tempfile.mkdtemp(
        prefix="cam_neffcache_")
    _os.environ["CAM_NEFF_CACHE_SET"] = "1"
    for _p in ("/var/tmp/neuron-compile-cache",
               _os.path.expanduser("~/.neuron-compile-cache")):
        _shutil.rmtree(_p, ignore_errors=True)

B, C, H, W = 16, 128, 128, 128
HW = H * W
N_CORES = 8
B_PER = B // N_CORES  # 2 samples per core
SCALE = 1.0 / float(np.sqrt(np.float32(HW)))  # 1/128

# tuning knobs (exp scripts override these before building variants)
CFG = {
    "store_engine": "sync",  # "sync" | "scalar" | "gpsimd"
    "att_copy": ("dve", "act"),  # per-block PSUM->SBUF copy engine cycle
    "pair_T": False,  # one [128,1024] PSUM->SBUF copy per super-group
    "ps_t": 4,
    "ps_a": 3,
    "xq_bufs": 12,
    "outp_bufs": 3,
    "gram_lead": 6,
    "vt_bufs": 9,
}

NQ = 4  # x quarter-loads per sample
QF = HW // NQ  # 4096 bf16 per quarter
ATT_N = 512  # attention matmul moving free dim (one PSUM bank)
OUT_BLK = 2048  # attention/output block width
SG = 1024  # gram super-group width (8 transposed chunks per PSUM bank)
N_SG = HW // SG  # 16 gram super-groups per sample
GRAM_LEAD = 2  # super-groups of transpose lead over the gram matmuls (default)
N_BLOCKS = HW // OUT_BLK  # 8 attention/output blocks
WARM = 8  # PE warm-up matmuls (cold-start clock ramp only)


class _SampleCtx:
    """Per-sample tiles threaded between the emission phases."""

    def __init__(self):
        self.xq = None
        self.eps = None
        self.sprime = None
        self.spT = None
        self.ot = None  # current [128, 2*OUT_BLK] output tile
        self.pt2 = None  # current [128, 2*SG] transpose PSUM pair tile
        self.vt = [None] * N_SG  # transposed bf16 super-group tiles


def _emit_load(nc, mybir, pools, x_d, b, sc, split_first=False):
    bf16 = mybir.dt.bfloat16
    xpool = pools["xpool"]
    sc.xq = []
    for q in range(NQ):
        xt = xpool.tile([128, QF], bf16, tag="xq")
        if pools.get("_mode") not in (None, "full", "dma"):
            # token write so the tile framework sees the tile as produced;
            # compute then reads (mostly garbage) SBUF with no DMA cost
            nc.sync.dma_start(out=xt[:, :16], in_=x_d[b, :, :16])
            sc.xq.append(xt)
            continue
        if q == 0 and split_first:
            # halve the first transfer so the gram phase starts earlier
            h = QF // 2
            nc.sync.dma_start(out=xt[:, :h], in_=x_d[b, :, :h])
            nc.sync.dma_start(out=xt[:, h:], in_=x_d[b, :, h:QF])
        else:
            nc.sync.dma_start(out=xt, in_=x_d[b, :, q * QF : (q + 1) * QF])
        sc.xq.append(xt)


def _emit_gram_T(nc, mybir, pools, sc, sg, engine="dve"):
    """Transpose half of one gram super-group: 8 PE transposes of [128,128]
    bf16 x chunks into half of a [128,2048] 2-bank PSUM tile.  On the odd
    sg of each pair, ONE [128,2048] PSUM->SBUF bf16 copy drains both banks
    (wider copies amortize the per-instruction fixed cost; DVE runs
    all-bf16 copies at 2x, ACT at 1x)."""
    bf16 = mybir.dt.bfloat16
    src = sc.xq[sg // (QF // SG)]
    scol = (sg % (QF // SG)) * SG
    pair = CFG.get("pair_T", True)
    w = 2 * SG if pair else SG
    if not pair or sg % 2 == 0:
        sc.pt2 = pools["ps_t"].tile([128, w], bf16, tag="pt")
    base = (sg % 2) * SG if pair else 0
    pt2 = sc.pt2
    for i in range(8):
        nc.tensor.matmul(
            pt2[:, base + i * 128 : base + (i + 1) * 128],
            src[:, scol + i * 128 : scol + (i + 1) * 128],
            pools["ident_bf16"],
            is_transpose=True,
            skip_group_check=True,
        )
    if not pair or sg % 2 == 1:
        vt = pools["vt"].tile([128, w], bf16)
        if engine == "act":
            nc.scalar.copy(vt, pt2)
        else:
            nc.vector.tensor_copy(vt, pt2)
        if pair:
            sc.vt[sg - 1] = (vt, 0)
            sc.vt[sg] = (vt, SG)
        else:
            sc.vt[sg] = (vt, 0)


def _emit_gram_MM(nc, mybir, pools, sc, sg):
    """Matmul half of one super-group: 8 accumulating vT.T @ vT matmuls."""
    f32 = mybir.dt.float32
    if sc.eps is None:
        sc.eps = pools["ps_g"].tile([128, 128], f32)
    vt, off = sc.vt[sg]
    for i in range(8):
        k = sg * 8 + i
        vti = vt[:, off + i * 128 : off + (i + 1) * 128]
        nc.tensor.matmul(
            sc.eps, vti, vti, start=(k == 0), stop=(k == 8 * N_SG - 1),
            skip_group_check=True,
        )
    sc.vt[sg] = None


def _emit_softmax_chain(nc, mybir, pools, sc):
    """Reversed softmax + gamma fold, ACT/DVE part (through sprime).

    Produces sprime = S' + I where S' = gamma * softmax_rev: with the
    identity folded in, the attention matmul computes gamma*(A @ V) + V
    directly, so the per-element residual add disappears (the
    V-passthrough is exact: 1.0 * bf16 V accumulated in f32 PSUM)."""
    f32 = mybir.dt.float32
    sm_pool = pools["sm"]
    eps = sc.eps
    rowmin = sm_pool.tile([128, 1], f32)
    nc.vector.tensor_reduce(
        rowmin, eps, axis=mybir.AxisListType.X, op=mybir.AluOpType.min
    )
    biasv = sm_pool.tile([128, 1], f32)
    nc.scalar.mul(biasv, rowmin, SCALE)
    p_sb = sm_pool.tile([128, 128], f32)
    zsum = sm_pool.tile([128, 1], f32)
    nc.scalar.activation(
        p_sb, eps, mybir.ActivationFunctionType.Exp,
        bias=biasv, scale=-SCALE, accum_out=zsum,
    )
    rz = sm_pool.tile([128, 1], f32)
    nc.vector.reciprocal(rz, zsum)
    rzg = sm_pool.tile([128, 1], f32)
    nc.vector.tensor_mul(rzg, rz, pools["gamma_sb"])
    # S' + I = (p * (gamma/Z)) + I in one fused DVE op
    sprime = sm_pool.tile([128, 128], f32)
    nc.vector.scalar_tensor_tensor(
        sprime, in0=p_sb, scalar=rzg, in1=pools["ident_f32"],
        op0=mybir.AluOpType.mult, op1=mybir.AluOpType.add,
    )
    sc.sprime = sprime


def _emit_softmax_T(nc, mybir, pools, sc):
    """PE transpose of sprime -> bf16 stationary (S' + I)^T."""
    f32 = mybir.dt.float32
    bf16 = mybir.dt.bfloat16
    pst = pools["ps_t"].tile([128, 512], f32, tag="pt")
    nc.tensor.matmul(pst[:, 0:128], sc.sprime, pools["ident_f32"],
                     is_transpose=True, skip_group_check=True)
    spT = pools["sm"].tile([128, 128], bf16)
    nc.vector.tensor_copy(spT, pst[:, 0:128])
    sc.spT = spT


def _emit_softmax(nc, mybir, pools, sc):
    _emit_softmax_chain(nc, mybir, pools, sc)
    _emit_softmax_T(nc, mybir, pools, sc)


def _emit_attn_block(nc, mybir, pools, y_d, b, sc, j, copy_engines=("act",),
                     store_engine="sync", ot_tile=None, do_store=True):
    """One [128, OUT_BLK] attention(+folded residual) block.  PSUM already
    holds the final y values ((S'+I) @ V); each [128, 2*ATT_N] PSUM tile is
    cast PSUM f32 -> SBUF bf16 by one ACT/DVE copy.  Output tiles span two
    blocks ([128, 2*OUT_BLK] bf16) so stores are 1MB DMAs; the DMA issues
    after the odd block of each pair.  store_engine="scalar" puts the store
    on the ACT HWDGE ring so it can't head-of-line-block loads on the SP
    ring."""
    bf16 = mybir.dt.bfloat16
    if j % 2 == 0:
        sc.ot = ot_tile if ot_tile is not None else pools["outp"].tile(
            [128, 2 * OUT_BLK], bf16, tag="ot")
    obase = (j % 2) * OUT_BLK
    xt = sc.xq[j // 2]
    xbase = (j % 2) * OUT_BLK
    for pp in range(OUT_BLK // ATT_N):  # N=512 chunks (one PSUM bank each)
        pa = pools["ps_a"].tile([128, ATT_N], mybir.dt.float32)
        off = xbase + pp * ATT_N
        nc.tensor.matmul(
            pa, sc.spT, xt[:, off : off + ATT_N], skip_group_check=True)
        osl = sc.ot[:, obase + pp * ATT_N : obase + (pp + 1) * ATT_N]
        if copy_engines[pp % len(copy_engines)] == "act":
            nc.scalar.copy(osl, pa)
        else:
            nc.vector.tensor_copy(osl, pa)
    if j % 2 == 1 and do_store and pools.get("_mode") != "nodma":
        eng = {"scalar": nc.scalar, "gpsimd": nc.gpsimd, "sync": nc.sync}[
            store_engine]
        eng.dma_start(
            out=y_d[b, :, (j - 1) * OUT_BLK : (j + 1) * OUT_BLK], in_=sc.ot)


def _emit_dma_only(nc, mybir, pools, x_d, y_d):
    """Loads + stores only (bandwidth floor measurement): store back the
    loaded x tiles (same transfer sizes/count as the real kernel)."""
    s0, s1 = _SampleCtx(), _SampleCtx()
    _emit_load(nc, mybir, pools, x_d, 0, s0, split_first=True)
    _emit_load(nc, mybir, pools, x_d, 1, s1)
    for b, sc in ((0, s0), (1, s1)):
        for q in range(NQ):
            nc.sync.dma_start(
                out=y_d[b, :, q * QF : (q + 1) * QF], in_=sc.xq[q])


def _emit_phased(nc, mybir, pools, x_d, y_d, mode):
    """Phase-isolation variants for attribution experiments.

    "t": transposes+copies only; "tg": + gram MMs + softmax;
    "attn": attention blocks only (fake stationary)."""
    bf16 = mybir.dt.bfloat16
    s0, s1 = _SampleCtx(), _SampleCtx()
    _emit_load(nc, mybir, pools, x_d, 0, s0, split_first=True)
    _emit_load(nc, mybir, pools, x_d, 1, s1)
    if mode in ("t", "tg"):
        for sc in (s0, s1):
            for sg in range(N_SG):
                _emit_gram_T(nc, mybir, pools, sc, sg,
                             engine="dve" if sg % 2 == 0 else "act")
                if mode == "tg" and sg >= GRAM_LEAD:
                    _emit_gram_MM(nc, mybir, pools, sc, sg - GRAM_LEAD)
            if mode == "tg":
                for sg in range(N_SG - GRAM_LEAD, N_SG):
                    _emit_gram_MM(nc, mybir, pools, sc, sg)
                _emit_softmax(nc, mybir, pools, sc)
    elif mode == "attn":
        for sc in (s0, s1):
            spT = pools["sm"].tile([128, 128], bf16)
            nc.scalar.copy(spT, pools["ident_bf16"])
            sc.spT = spT
            for j in range(N_BLOCKS):
                _emit_attn_block(nc, mybir, pools, y_d, 0, sc, j,
                                 copy_engines=("act", "dve"))
    elif mode == "tno":
        # pure PE transpose rate: 256 transposes, no PSUM->SBUF copies
        for sc in (s0, s1):
            for sg in range(N_SG):
                src = sc.xq[sg // (QF // SG)]
                scol = (sg % (QF // SG)) * SG
                pt = pools["ps_t"].tile([128, SG], bf16, tag="pt")
                for i in range(8):
                    nc.tensor.matmul(
                        pt[:, i * 128 : (i + 1) * 128],
                        src[:, scol + i * 128 : scol + (i + 1) * 128],
                        pools["ident_bf16"],
                        is_transpose=True,
                        skip_group_check=True,
                    )
    elif mode == "gno":
        # pure LDW+MM rate: 256 accumulating gram matmuls, stationary
        # alternates between two static tiles (forces a weight load each)
        va = pools["sm"].tile([128, 128], bf16)
        vb = pools["sm"].tile([128, 128], bf16)
        nc.scalar.copy(va, pools["ident_bf16"])
        nc.scalar.copy(vb, pools["ident_bf16"])
        f32 = mybir.dt.float32
        for s in range(B_PER):
            eps = pools["ps_g"].tile([128, 128], f32)
            for k in range(8 * N_SG):
                vt = va if k % 2 == 0 else vb
                nc.tensor.matmul(
                    eps, vt, vt, start=(k == 0), stop=(k == 8 * N_SG - 1),
                    skip_group_check=True,
                )
    elif mode == "ano":
        # pure attention MM rate: 64 N=512 MMs, fixed stationary, no copies
        spT = pools["sm"].tile([128, 128], bf16)
        nc.scalar.copy(spT, pools["ident_bf16"])
        for sc in (s0, s1):
            for j in range(N_BLOCKS):
                xt = sc.xq[j // 2]
                xbase = (j % 2) * OUT_BLK
                for pp in range(OUT_BLK // ATT_N):
                    pa = pools["ps_a"].tile([128, ATT_N], mybir.dt.float32)
                    off = xbase + pp * ATT_N
                    nc.tensor.matmul(
                        pa, spT, xt[:, off : off + ATT_N],
                        skip_group_check=True)
    else:
        raise ValueError(mode)


def _emit_workload(nc, mybir, pools, x_d, y_d, mode="full", carried=None):
    """Both samples, software-pipelined in emission order."""
    if mode == "dma":
        return _emit_dma_only(nc, mybir, pools, x_d, y_d)
    if mode not in ("full", "nodma"):
        return _emit_phased(nc, mybir, pools, x_d, y_d, mode)
    f32 = mybir.dt.float32
    s0, s1 = _SampleCtx(), _SampleCtx()

    # PE warm-up: a few dependency-free matmuls during the load head help
    # the cold-start clock ramp; kept short because in the steady-state
    # rep loop they are pure overhead (PE is already hot).
    warm = pools["ps_t"].tile([128, 128], f32, tag="pt")
    for w in range(WARM):
        nc.tensor.matmul(warm, pools["ident_bf16"], pools["ident_bf16"],
                         skip_group_check=True)

    if carried is None:
        _emit_load(nc, mybir, pools, x_d, 0, s0, split_first=True)
        _emit_load(nc, mybir, pools, x_d, 1, s1)
    else:
        # Deferred-store pipeline (rep-loop body): the previous body's
        # sample-1 output tiles are stored HERE, interleaved between this
        # body's loads on the same sync HWDGE ring.  This keeps the DMA
        # stream gapless across the loop back-edge: without it the next
        # body's loads queue behind ALL of this body's stores in ring
        # order, serializing the pipeline at ~(stores tail + loads head).
        bf16 = mybir.dt.bfloat16
        xpool = pools["xpool"]
        s0.xq, s1.xq = [], []
        for sc in (s0, s1):
            for q in range(NQ):
                xt = xpool.tile([128, QF], bf16, tag="xq")
                sc.xq.append(xt)

        def _ld(sc, b, q):
            nc.sync.dma_start(out=sc.xq[q], in_=x_d[b, :, q * QF:(q + 1) * QF])

        def _st(p):
            nc.sync.dma_start(
                out=y_d[1, :, p * 2 * OUT_BLK:(p + 1) * 2 * OUT_BLK],
                in_=carried[p])

        h = QF // 2
        nc.sync.dma_start(out=s0.xq[0][:, :h], in_=x_d[0, :, :h])
        nc.sync.dma_start(out=s0.xq[0][:, h:], in_=x_d[0, :, h:QF])
        _ld(s0, 0, 1)
        _st(0)
        _ld(s0, 0, 2)
        _st(1)
        _ld(s0, 0, 3)
        _st(2)
        _ld(s1, 1, 0)
        _st(3)
        _ld(s1, 1, 1)
        _ld(s1, 1, 2)
        _ld(s1, 1, 3)

    # gram copy engine pattern: 2/3 DVE (bf16 copies run 2x there), 1/3 ACT
    def _geng(sg):
        return "act" if sg % 3 == 2 else "dve"

    # sample-0 gram runs alone, software-pipelined with `lead` super-groups
    # of transpose lead so the PSUM->SBUF copy round-trip of super-group g
    # hides behind the transposes of g+1..g+lead.
    lead = CFG.get("gram_lead", GRAM_LEAD)
    for sg in range(N_SG):
        _emit_gram_T(nc, mybir, pools, s0, sg, engine=_geng(sg))
        if sg >= lead:
            _emit_gram_MM(nc, mybir, pools, s0, sg - lead)
    for sg in range(N_SG - lead, N_SG):
        _emit_gram_MM(nc, mybir, pools, s0, sg)

    # softmax-0 ACT/DVE chain runs while the PE starts sample-1 transposes
    _emit_softmax_chain(nc, mybir, pools, s0)
    _emit_gram_T(nc, mybir, pools, s1, 0, engine=_geng(0))
    _emit_gram_T(nc, mybir, pools, s1, 1, engine=_geng(1))
    _emit_softmax_T(nc, mybir, pools, s0)

    # interleave: sample-0 attention blocks (j=0..6) with the remaining
    # sample-1 gram super-groups.  A-copies alternate ACT/DVE.
    t_idx, mm_idx = 2, 0
    for j in range(N_BLOCKS - 1):
        _emit_attn_block(nc, mybir, pools, y_d, 0, s0, j,
                         copy_engines=CFG["att_copy"], store_engine=CFG["store_engine"])
        for _ in range(2):
            if t_idx < N_SG:
                _emit_gram_T(nc, mybir, pools, s1, t_idx, engine=_geng(t_idx))
                t_idx += 1
            if mm_idx < t_idx - lead and mm_idx < N_SG:
                _emit_gram_MM(nc, mybir, pools, s1, mm_idx)
                mm_idx += 1
    while mm_idx < N_SG:
        _emit_gram_MM(nc, mybir, pools, s1, mm_idx)
        mm_idx += 1

    # softmax-1 chain overlaps the last sample-0 attention block on the PE
    _emit_softmax_chain(nc, mybir, pools, s1)
    _emit_attn_block(nc, mybir, pools, y_d, 0, s0, N_BLOCKS - 1,
                     copy_engines=CFG["att_copy"], store_engine=CFG["store_engine"])
    _emit_softmax_T(nc, mybir, pools, s1)

    for j in range(N_BLOCKS):
        _emit_attn_block(
            nc, mybir, pools, y_d, 1, s1, j,
            copy_engines=CFG["att_copy"], store_engine=CFG["store_engine"],
            ot_tile=(carried[j // 2] if (carried is not None and j % 2 == 0)
                     else None),
            do_store=(carried is None))


def _build_bass(reps=0, unroll=1, mode="full", flat=1):
    """Build the Bass program. reps>0 wraps the workload in a HW loop that
    repeats it (for steady-state benchmarking; output is idempotent);
    unroll>1 amortizes the loop back-edge (barrier + IRAM refetch).
    mode: "full" | "dma" (loads+stores only) | "nodma" (compute only)."""
    import concourse.bacc as bacc
    import concourse.tile as tile
    from concourse import masks, mybir
    from contextlib import ExitStack

    f32 = mybir.dt.float32
    bf16 = mybir.dt.bfloat16

    # Bacc (not plain Bass): its compile() runs generate_event_semaphores,
    # which splits multi-wait instructions — walrus rejects them on TRN2.
    nc = bacc.Bacc(
        "TRN2",
        target_bir_lowering=False,
        debug=False,
        enable_asserts=False,
        num_devices=N_CORES,
    )
    x_d = nc.dram_tensor("x", [B_PER, C, HW], bf16, kind="ExternalInput")
    g_d = nc.dram_tensor("gamma", [1], f32, kind="ExternalInput")
    y_d = nc.dram_tensor("y", [B_PER, C, HW], bf16, kind="ExternalOutput")

    with tile.TileContext(nc) as tc, ExitStack() as ctx:
        pools = {}
        for name, kw in [
            ("consts", dict(bufs=1)),
            ("xpool", dict(bufs=CFG["xq_bufs"])),
            ("vt", dict(bufs=CFG.get("vt_bufs", 6))),
            ("sm", dict(bufs=4)),
            ("outp", dict(bufs=CFG["outp_bufs"])),
            ("ps_t", dict(bufs=CFG["ps_t"], space="PSUM")),  # [128,1024] bf16: 1 bank each
            ("ps_g", dict(bufs=1, space="PSUM")),
            ("ps_a", dict(bufs=CFG["ps_a"], space="PSUM")),  # [128,512] f32: 1 bank each
        ]:
            pools[name] = ctx.enter_context(tc.tile_pool(name=name, **kw))

        ident_f32 = pools["consts"].tile([128, 128], f32)
        masks.make_identity(nc, ident_f32)
        ident_bf16 = pools["consts"].tile([128, 128], mybir.dt.bfloat16)
        masks.make_identity(nc, ident_bf16)
        gamma_sb = pools["consts"].tile([128, 1], f32)
        nc.gpsimd.dma_start(out=gamma_sb, in_=g_d[:].to_broadcast((128, 1)))
        pools["ident_f32"] = ident_f32
        pools["ident_bf16"] = ident_bf16
        pools["gamma_sb"] = gamma_sb

        pools["_mode"] = mode
        if flat > 1:
            # loop-free repetition for TimelineSim period analysis
            carried = None
            if mode == "full":
                outd = ctx.enter_context(tc.tile_pool(name="outd", bufs=1))
                carried = []
                for p in range(4):
                    t = outd.tile([128, 2 * OUT_BLK], bf16, tag=f"od{p}")
                    nc.gpsimd.memset(t, 0)
                    carried.append(t)
            for _ in range(flat):
                _emit_workload(nc, mybir, pools, x_d, y_d, mode=mode,
                               carried=carried)
        elif reps:
            carried = None
            if mode == "full":
                # persistent sample-1 output tiles for the deferred-store
                # pipeline; zero-filled so iteration 0's deferred stores
                # read initialized data (overwritten by iterations >= 1)
                outd = ctx.enter_context(tc.tile_pool(name="outd", bufs=1))
                carried = []
                for p in range(4):
                    t = outd.tile([128, 2 * OUT_BLK], bf16, tag=f"od{p}")
                    nc.gpsimd.memset(t, 0)
                    carried.append(t)
            # PE body is several hundred instructions (> 1 IRAM block):
            # hint the back-edge prefetch so the bench loop doesn't pay an
            # I$ miss.
            with tc.For_i(0, reps, 1, hint_engines=(mybir.EngineType.PE,)):
                for _ in range(unroll):
                    _emit_workload(nc, mybir, pools, x_d, y_d, mode=mode,
                                   carried=carried)
        else:
            _emit_workload(nc, mybir, pools, x_d, y_d, mode=mode)

    nc.compile()
    return nc


_NC_CACHE = None


def _get_nc():
    global _NC_CACHE
    if _NC_CACHE is None:
        _NC_CACHE = _build_bass()
    return _NC_CACHE


def kernel(x, gamma, trace=False):
    from concourse.bass_utils import run_bass_kernel_spmd
    from concourse import mybir

    np_bf16 = mybir.dt.np(mybir.dt.bfloat16)
    x = np.asarray(x, dtype=np.float32).astype(np_bf16)
    gamma = np.asarray(gamma, dtype=np.float32)
    nc = _get_nc()

    xs = x.reshape(N_CORES, B_PER, C, HW)
    in_maps = [{"x": xs[i], "gamma": gamma} for i in range(N_CORES)]
    res = run_bass_kernel_spmd(nc, in_maps, core_ids=list(range(N_CORES)), trace=trace)
    out = np.stack([res.results[i]["y"] for i in range(N_CORES)], axis=0)
    out = out.astype(np.float32).reshape(B, C, H, W)
    if trace:
        return out, res
    return out



# revision 40
# speedup vs baseline: 1.0890x; 1.0105x over previous
"""CAM (channel attention module) Bass kernel for Trainium2.

Problem: y = gamma * (softmax_rev(v @ v.T * s) @ v) + x per batch sample,
with x [16, 128, 128, 128] f32, v = x.reshape(B, C, H*W).

Sharding: pure data parallel — B=16 split as 2 samples per core across
8 NeuronCores; gamma replicated; no collectives.

HBM traffic strategy: the module is memory-bound, so both x and y cross
HBM as bf16 (host casts f32<->bf16 around the device call).  Compute was
already all-bf16 on the PE in the f32-I/O version, so the only added
error is bf16 rounding of the residual x and of the stored y — ~0.2% of
per-element magnitude each, far inside the 2e-2 gate.  This halves the
HBM bytes per core (33.6 MB -> 16.8 MB) and deletes the entire bf16
rhs-copy production stage of the f32 version (x arrives in SBUF already
bf16, so gram transposes, attention rhs, and the residual add all read
the loaded x tiles directly).

Per-core dataflow (per sample, [C=128, HW=16384] bf16):
  1. DMA both samples into SBUF (quarter-loads, all on the sync HWDGE
     ring).  In the rep-loop body the previous body's sample-1 stores are
     interleaved BETWEEN these loads (deferred-store pipeline, persistent
     carried output tiles) so the DMA ring never serializes
     loads-behind-stores across the loop back-edge.
  2. Gram matrix E = V V^T: PE transposes bf16 chunks (8 per PSUM bank),
     one ACT/DVE copy PSUM->SBUF per super-group (2/3 DVE: bf16 copies
     run 2x there), PE accumulates vT.T @ vT into a PSUM bank with
     gram_lead=6 super-groups of transpose lead so matmuls never wait on
     the copy round-trip.
  3. Reversed softmax: rowmin of E (DVE), p = exp(-s*E + s*rowmin) with
     fused row-sum Z (single ACT op), r = 1/Z (DVE), fold gamma: S' =
     p * (gamma*r) per row; PE-transpose -> bf16 stationary S'T.  The
     ACT/DVE chain of each softmax is emitted so it overlaps PE work of
     the other sample (first transposes of s1, last attention block of
     s0).
  4. Attention: psum = S'T.T @ x_chunk (32 matmuls, N=512); PSUM already
     holds final y (residual folded via identity in S'T); copies
     alternate DVE/ACT -> batched 1MB bf16 DMA out.
  Sample 0's attention phase is interleaved with sample 1's Gram phase in
  emission order so the PE/ACT/DVE streams of the two samples overlap.

Measured on the 8-core bench: 59.1 us/rep (baseline 69.1; DMA-only floor
for the 16.8 MB/core of traffic is ~47-50 us, compute-only ~56-59 us, so
the kernel is compute-pipeline-bound and sits ~6 us above the pure-DMA
floor).
"""

import os as _os
import shutil as _shutil
import tempfile as _tempfile

import numpy as np

# The libneuronxla NEFF cache key does not cover the Bass BIR embedded in
# the jit custom call: two different Bass programs with the same outer HLO
# (same shapes/dtypes) collide, silently serving the wrong NEFF. Point the
# cache at a private fresh dir before the first compile in this process,
# and drop any pre-existing default caches.
if not _os.environ.get("CAM_NEFF_CACHE_SET"):
    _os.environ["NEURON_COMPILE_CACHE_URL"] = _tempfile.mkdtemp(
        prefix="cam_neffcache_")
    _os.environ["CAM_NEFF_CACHE_SET"] = "1"
    for _p in ("/var/tmp/neuron-compile-cache",
               _os.path.expanduser("~/.neuron-compile-cache")):
        _shutil.rmtree(_p, ignore_errors=True)

B, C, H, W = 16, 128, 128, 128
HW = H * W
N_CORES = 8
B_PER = B // N_CORES  # 2 samples per core
SCALE = 1.0 / float(np.sqrt(np.float32(HW)))  # 1/128

# tuning knobs (exp scripts override these before building variants)
CFG = {
    "store_engine": "sync",  # "sync" | "scalar" | "gpsimd"
    "att_copy": ("dve", "act"),  # per-block PSUM->SBUF copy engine cycle
    "pair_T": False,  # one [128,1024] PSUM->SBUF copy per super-group
    "ps_t": 4,
    "ps_a": 3,
    "xq_bufs": 12,
    "outp_bufs": 3,
    "gram_lead": 6,
    "vt_bufs": 9,
}

NQ = 4  # x quarter-loads per sample
QF = HW // NQ  # 4096 bf16 per quarter
ATT_N = 512  # attention matmul moving free dim (one PSUM bank)
OUT_BLK = 2048  # attention/output block width
SG = 1024  # gram super-group width (8 transposed chunks per PSUM bank)
N_SG = HW // SG  # 16 gram super-groups per sample
GRAM_LEAD = 2  # super-groups of transpose lead over the gram matmuls (default)
N_BLOCKS = HW // OUT_BLK  # 8 attention/output blocks
WARM = 8  # PE warm-up matmuls (cold-start clock ramp only)


class _SampleCtx:
    """Per-sample tiles threaded between the emission phases."""

    def __init__(self):
        self.xq = None
        self.eps = None
        self.sprime = None
        self.spT = None
        self.ot = None  # current [128, 2*OUT_BLK] output tile
        self.pt2 = None  # current [128, 2*SG] transpose PSUM pair tile
        self.vt = [None] * N_SG  # transposed bf16 super-group tiles


def _emit_load(nc, mybir, pools, x_d, b, sc, split_first=False):
    bf16 = mybir.dt.bfloat16
    xpool = pools["xpool"]
    sc.xq = []
    for q in range(NQ):
        xt = xpool.tile([128, QF], bf16, tag="xq")
        if pools.get("_mode") not in (None, "full", "dma"):
            # token write so the tile framework sees the tile as produced;
            # compute then reads (mostly garbage) SBUF with no DMA cost
            nc.sync.dma_start(out=xt[:, :16], in_=x_d[b, :, :16])
            sc.xq.append(xt)
            continue
        if q == 0 and split_first:
            # halve the first transfer so the gram phase starts earlier
            h = QF // 2
            nc.sync.dma_start(out=xt[:, :h], in_=x_d[b, :, :h])
            nc.sync.dma_start(out=xt[:, h:], in_=x_d[b, :, h:QF])
        else:
            nc.sync.dma_start(out=xt, in_=x_d[b, :, q * QF : (q + 1) * QF])
        sc.xq.append(xt)


def _emit_gram_T(nc, mybir, pools, sc, sg, engine="dve"):
    """Transpose half of one gram super-group: 8 PE transposes of [128,128]
    bf16 x chunks into a PSUM tile, then one PSUM->SBUF bf16 copy (DVE
    runs all-bf16 copies at 2x; ACT at 1x).  CFG["pair_T"] selects
    [128,2048] 2-bank pair tiles with one copy per two super-groups
    (measured slower: fewer PSUM tiles in rotation stall the transposes),
    default is one [128,1024] bank + copy per super-group."""
    bf16 = mybir.dt.bfloat16
    src = sc.xq[sg // (QF // SG)]
    scol = (sg % (QF // SG)) * SG
    pair = CFG.get("pair_T", True)
    w = 2 * SG if pair else SG
    if not pair or sg % 2 == 0:
        sc.pt2 = pools["ps_t"].tile([128, w], bf16, tag="pt")
    base = (sg % 2) * SG if pair else 0
    pt2 = sc.pt2
    for i in range(8):
        nc.tensor.matmul(
            pt2[:, base + i * 128 : base + (i + 1) * 128],
            src[:, scol + i * 128 : scol + (i + 1) * 128],
            pools["ident_bf16"],
            is_transpose=True,
            skip_group_check=True,
        )
    if not pair or sg % 2 == 1:
        vt = pools["vt"].tile([128, w], bf16)
        if engine == "act":
            nc.scalar.copy(vt, pt2)
        else:
            nc.vector.tensor_copy(vt, pt2)
        if pair:
            sc.vt[sg - 1] = (vt, 0)
            sc.vt[sg] = (vt, SG)
        else:
            sc.vt[sg] = (vt, 0)


def _emit_gram_MM(nc, mybir, pools, sc, sg):
    """Matmul half of one super-group: 8 accumulating vT.T @ vT matmuls."""
    f32 = mybir.dt.float32
    if sc.eps is None:
        sc.eps = pools["ps_g"].tile([128, 128], f32)
    vt, off = sc.vt[sg]
    for i in range(8):
        k = sg * 8 + i
        vti = vt[:, off + i * 128 : off + (i + 1) * 128]
        nc.tensor.matmul(
            sc.eps, vti, vti, start=(k == 0), stop=(k == 8 * N_SG - 1),
            skip_group_check=True,
        )
    sc.vt[sg] = None


def _emit_softmax_chain(nc, mybir, pools, sc):
    """Reversed softmax + gamma fold, ACT/DVE part (through sprime).

    Produces sprime = S' + I where S' = gamma * softmax_rev: with the
    identity folded in, the attention matmul computes gamma*(A @ V) + V
    directly, so the per-element residual add disappears (the
    V-passthrough is exact: 1.0 * bf16 V accumulated in f32 PSUM)."""
    f32 = mybir.dt.float32
    sm_pool = pools["sm"]
    eps = sc.eps
    rowmin = sm_pool.tile([128, 1], f32)
    nc.vector.tensor_reduce(
        rowmin, eps, axis=mybir.AxisListType.X, op=mybir.AluOpType.min
    )
    biasv = sm_pool.tile([128, 1], f32)
    nc.scalar.mul(biasv, rowmin, SCALE)
    p_sb = sm_pool.tile([128, 128], f32)
    zsum = sm_pool.tile([128, 1], f32)
    nc.scalar.activation(
        p_sb, eps, mybir.ActivationFunctionType.Exp,
        bias=biasv, scale=-SCALE, accum_out=zsum,
    )
    rz = sm_pool.tile([128, 1], f32)
    nc.vector.reciprocal(rz, zsum)
    rzg = sm_pool.tile([128, 1], f32)
    nc.vector.tensor_mul(rzg, rz, pools["gamma_sb"])
    # S' + I = (p * (gamma/Z)) + I in one fused DVE op
    sprime = sm_pool.tile([128, 128], f32)
    nc.vector.scalar_tensor_tensor(
        sprime, in0=p_sb, scalar=rzg, in1=pools["ident_f32"],
        op0=mybir.AluOpType.mult, op1=mybir.AluOpType.add,
    )
    sc.sprime = sprime


def _emit_softmax_T(nc, mybir, pools, sc):
    """PE transpose of sprime -> bf16 stationary (S' + I)^T."""
    f32 = mybir.dt.float32
    bf16 = mybir.dt.bfloat16
    pst = pools["ps_t"].tile([128, 512], f32, tag="pt")
    nc.tensor.matmul(pst[:, 0:128], sc.sprime, pools["ident_f32"],
                     is_transpose=True, skip_group_check=True)
    spT = pools["sm"].tile([128, 128], bf16)
    nc.vector.tensor_copy(spT, pst[:, 0:128])
    sc.spT = spT


def _emit_softmax(nc, mybir, pools, sc):
    _emit_softmax_chain(nc, mybir, pools, sc)
    _emit_softmax_T(nc, mybir, pools, sc)


def _emit_attn_block(nc, mybir, pools, y_d, b, sc, j, copy_engines=("act",),
                     store_engine="sync", ot_tile=None, do_store=True):
    """One [128, OUT_BLK] attention(+folded residual) block.  PSUM already
    holds the final y values ((S'+I) @ V); each [128, 2*ATT_N] PSUM tile is
    cast PSUM f32 -> SBUF bf16 by one ACT/DVE copy.  Output tiles span two
    blocks ([128, 2*OUT_BLK] bf16) so stores are 1MB DMAs; the DMA issues
    after the odd block of each pair.  store_engine="scalar" puts the store
    on the ACT HWDGE ring so it can't head-of-line-block loads on the SP
    ring."""
    bf16 = mybir.dt.bfloat16
    if j % 2 == 0:
        sc.ot = ot_tile if ot_tile is not None else pools["outp"].tile(
            [128, 2 * OUT_BLK], bf16, tag="ot")
    obase = (j % 2) * OUT_BLK
    xt = sc.xq[j // 2]
    xbase = (j % 2) * OUT_BLK
    for pp in range(OUT_BLK // ATT_N):  # N=512 chunks (one PSUM bank each)
        pa = pools["ps_a"].tile([128, ATT_N], mybir.dt.float32)
        off = xbase + pp * ATT_N
        nc.tensor.matmul(
            pa, sc.spT, xt[:, off : off + ATT_N], skip_group_check=True)
        osl = sc.ot[:, obase + pp * ATT_N : obase + (pp + 1) * ATT_N]
        if copy_engines[pp % len(copy_engines)] == "act":
            nc.scalar.copy(osl, pa)
        else:
            nc.vector.tensor_copy(osl, pa)
    if j % 2 == 1 and do_store and pools.get("_mode") != "nodma":
        eng = {"scalar": nc.scalar, "gpsimd": nc.gpsimd, "sync": nc.sync}[
            store_engine]
        eng.dma_start(
            out=y_d[b, :, (j - 1) * OUT_BLK : (j + 1) * OUT_BLK], in_=sc.ot)


def _emit_dma_only(nc, mybir, pools, x_d, y_d):
    """Loads + stores only (bandwidth floor measurement): store back the
    loaded x tiles (same transfer sizes/count as the real kernel)."""
    s0, s1 = _SampleCtx(), _SampleCtx()
    _emit_load(nc, mybir, pools, x_d, 0, s0, split_first=True)
    _emit_load(nc, mybir, pools, x_d, 1, s1)
    for b, sc in ((0, s0), (1, s1)):
        for q in range(NQ):
            nc.sync.dma_start(
                out=y_d[b, :, q * QF : (q + 1) * QF], in_=sc.xq[q])


def _emit_phased(nc, mybir, pools, x_d, y_d, mode):
    """Phase-isolation variants for attribution experiments.

    "t": transposes+copies only; "tg": + gram MMs + softmax;
    "attn": attention blocks only (fake stationary)."""
    bf16 = mybir.dt.bfloat16
    s0, s1 = _SampleCtx(), _SampleCtx()
    _emit_load(nc, mybir, pools, x_d, 0, s0, split_first=True)
    _emit_load(nc, mybir, pools, x_d, 1, s1)
    if mode in ("t", "tg"):
        for sc in (s0, s1):
            for sg in range(N_SG):
                _emit_gram_T(nc, mybir, pools, sc, sg,
                             engine="dve" if sg % 2 == 0 else "act")
                if mode == "tg" and sg >= GRAM_LEAD:
                    _emit_gram_MM(nc, mybir, pools, sc, sg - GRAM_LEAD)
            if mode == "tg":
                for sg in range(N_SG - GRAM_LEAD, N_SG):
                    _emit_gram_MM(nc, mybir, pools, sc, sg)
                _emit_softmax(nc, mybir, pools, sc)
    elif mode == "attn":
        for sc in (s0, s1):
            spT = pools["sm"].tile([128, 128], bf16)
            nc.scalar.copy(spT, pools["ident_bf16"])
            sc.spT = spT
            for j in range(N_BLOCKS):
                _emit_attn_block(nc, mybir, pools, y_d, 0, sc, j,
                                 copy_engines=("act", "dve"))
    elif mode == "tno":
        # pure PE transpose rate: 256 transposes, no PSUM->SBUF copies
        for sc in (s0, s1):
            for sg in range(N_SG):
                src = sc.xq[sg // (QF // SG)]
                scol = (sg % (QF // SG)) * SG
                pt = pools["ps_t"].tile([128, SG], bf16, tag="pt")
                for i in range(8):
                    nc.tensor.matmul(
                        pt[:, i * 128 : (i + 1) * 128],
                        src[:, scol + i * 128 : scol + (i + 1) * 128],
                        pools["ident_bf16"],
                        is_transpose=True,
                        skip_group_check=True,
                    )
    elif mode == "gno":
        # pure LDW+MM rate: 256 accumulating gram matmuls, stationary
        # alternates between two static tiles (forces a weight load each)
        va = pools["sm"].tile([128, 128], bf16)
        vb = pools["sm"].tile([128, 128], bf16)
        nc.scalar.copy(va, pools["ident_bf16"])
        nc.scalar.copy(vb, pools["ident_bf16"])
        f32 = mybir.dt.float32
        for s in range(B_PER):
            eps = pools["ps_g"].tile([128, 128], f32)
            for k in range(8 * N_SG):
                vt = va if k % 2 == 0 else vb
                nc.tensor.matmul(
                    eps, vt, vt, start=(k == 0), stop=(k == 8 * N_SG - 1),
                    skip_group_check=True,
                )
    elif mode == "ano":
        # pure attention MM rate: 64 N=512 MMs, fixed stationary, no copies
        spT = pools["sm"].tile([128, 128], bf16)
        nc.scalar.copy(spT, pools["ident_bf16"])
        for sc in (s0, s1):
            for j in range(N_BLOCKS):
                xt = sc.xq[j // 2]
                xbase = (j % 2) * OUT_BLK
                for pp in range(OUT_BLK // ATT_N):
                    pa = pools["ps_a"].tile([128, ATT_N], mybir.dt.float32)
                    off = xbase + pp * ATT_N
                    nc.tensor.matmul(
                        pa, spT, xt[:, off : off + ATT_N],
                        skip_group_check=True)
    else:
        raise ValueError(mode)


def _emit_workload(nc, mybir, pools, x_d, y_d, mode="full", carried=None):
    """Both samples, software-pipelined in emission order."""
    if mode == "dma":
        return _emit_dma_only(nc, mybir, pools, x_d, y_d)
    if mode not in ("full", "nodma"):
        return _emit_phased(nc, mybir, pools, x_d, y_d, mode)
    f32 = mybir.dt.float32
    s0, s1 = _SampleCtx(), _SampleCtx()

    # PE warm-up: a few dependency-free matmuls during the load head help
    # the cold-start clock ramp; kept short because in the steady-state
    # rep loop they are pure overhead (PE is already hot).
    warm = pools["ps_t"].tile([128, 128], f32, tag="pt")
    for w in range(WARM):
        nc.tensor.matmul(warm, pools["ident_bf16"], pools["ident_bf16"],
                         skip_group_check=True)

    if carried is None:
        _emit_load(nc, mybir, pools, x_d, 0, s0, split_first=True)
        _emit_load(nc, mybir, pools, x_d, 1, s1)
    else:
        # Deferred-store pipeline (rep-loop body): the previous body's
        # sample-1 output tiles are stored HERE, interleaved between this
        # body's loads on the same sync HWDGE ring.  This keeps the DMA
        # stream gapless across the loop back-edge: without it the next
        # body's loads queue behind ALL of this body's stores in ring
        # order, serializing the pipeline at ~(stores tail + loads head).
        bf16 = mybir.dt.bfloat16
        xpool = pools["xpool"]
        s0.xq, s1.xq = [], []
        for sc in (s0, s1):
            for q in range(NQ):
                xt = xpool.tile([128, QF], bf16, tag="xq")
                sc.xq.append(xt)

        def _ld(sc, b, q):
            nc.sync.dma_start(out=sc.xq[q], in_=x_d[b, :, q * QF:(q + 1) * QF])

        def _st(p):
            nc.sync.dma_start(
                out=y_d[1, :, p * 2 * OUT_BLK:(p + 1) * 2 * OUT_BLK],
                in_=carried[p])

        h = QF // 2
        nc.sync.dma_start(out=s0.xq[0][:, :h], in_=x_d[0, :, :h])
        nc.sync.dma_start(out=s0.xq[0][:, h:], in_=x_d[0, :, h:QF])
        _ld(s0, 0, 1)
        _st(0)
        _ld(s0, 0, 2)
        _st(1)
        _ld(s0, 0, 3)
        _st(2)
        _ld(s1, 1, 0)
        _st(3)
        _ld(s1, 1, 1)
        _ld(s1, 1, 2)
        _ld(s1, 1, 3)

    # gram copy engine pattern: 2/3 DVE (bf16 copies run 2x there), 1/3 ACT
    def _geng(sg):
        return "act" if sg % 3 == 2 else "dve"

    # sample-0 gram runs alone, software-pipelined with `lead` super-groups
    # of transpose lead so the PSUM->SBUF copy round-trip of super-group g
    # hides behind the transposes of g+1..g+lead.
    lead = CFG.get("gram_lead", GRAM_LEAD)
    for sg in range(N_SG):
        _emit_gram_T(nc, mybir, pools, s0, sg, engine=_geng(sg))
        if sg >= lead:
            _emit_gram_MM(nc, mybir, pools, s0, sg - lead)
    for sg in range(N_SG - lead, N_SG):
        _emit_gram_MM(nc, mybir, pools, s0, sg)

    # softmax-0 ACT/DVE chain runs while the PE starts sample-1 transposes
    _emit_softmax_chain(nc, mybir, pools, s0)
    _emit_gram_T(nc, mybir, pools, s1, 0, engine=_geng(0))
    _emit_gram_T(nc, mybir, pools, s1, 1, engine=_geng(1))
    _emit_softmax_T(nc, mybir, pools, s0)

    # interleave: sample-0 attention blocks (j=0..6) with the remaining
    # sample-1 gram super-groups.  A-copies alternate ACT/DVE.
    t_idx, mm_idx = 2, 0
    for j in range(N_BLOCKS - 1):
        _emit_attn_block(nc, mybir, pools, y_d, 0, s0, j,
                         copy_engines=CFG["att_copy"], store_engine=CFG["store_engine"])
        for _ in range(2):
            if t_idx < N_SG:
                _emit_gram_T(nc, mybir, pools, s1, t_idx, engine=_geng(t_idx))
                t_idx += 1
            if mm_idx < t_idx - lead and mm_idx < N_SG:
                _emit_gram_MM(nc, mybir, pools, s1, mm_idx)
                mm_idx += 1
    while mm_idx < N_SG:
        _emit_gram_MM(nc, mybir, pools, s1, mm_idx)
        mm_idx += 1

    # softmax-1 chain overlaps the last sample-0 attention block on the PE
    _emit_softmax_chain(nc, mybir, pools, s1)
    _emit_attn_block(nc, mybir, pools, y_d, 0, s0, N_BLOCKS - 1,
                     copy_engines=CFG["att_copy"], store_engine=CFG["store_engine"])
    _emit_softmax_T(nc, mybir, pools, s1)

    for j in range(N_BLOCKS):
        _emit_attn_block(
            nc, mybir, pools, y_d, 1, s1, j,
            copy_engines=CFG["att_copy"], store_engine=CFG["store_engine"],
            ot_tile=(carried[j // 2] if (carried is not None and j % 2 == 0)
                     else None),
            do_store=(carried is None))


def _build_bass(reps=0, unroll=1, mode="full", flat=1):
    """Build the Bass program. reps>0 wraps the workload in a HW loop that
    repeats it (for steady-state benchmarking; output is idempotent);
    unroll>1 amortizes the loop back-edge (barrier + IRAM refetch).
    mode: "full" | "dma" (loads+stores only) | "nodma" (compute only)."""
    import concourse.bacc as bacc
    import concourse.tile as tile
    from concourse import masks, mybir
    from contextlib import ExitStack

    f32 = mybir.dt.float32
    bf16 = mybir.dt.bfloat16

    # Bacc (not plain Bass): its compile() runs generate_event_semaphores,
    # which splits multi-wait instructions — walrus rejects them on TRN2.
    nc = bacc.Bacc(
        "TRN2",
        target_bir_lowering=False,
        debug=False,
        enable_asserts=False,
        num_devices=N_CORES,
    )
    x_d = nc.dram_tensor("x", [B_PER, C, HW], bf16, kind="ExternalInput")
    g_d = nc.dram_tensor("gamma", [1], f32, kind="ExternalInput")
    y_d = nc.dram_tensor("y", [B_PER, C, HW], bf16, kind="ExternalOutput")

    with tile.TileContext(nc) as tc, ExitStack() as ctx:
        pools = {}
        for name, kw in [
            ("consts", dict(bufs=1)),
            ("xpool", dict(bufs=CFG["xq_bufs"])),
            ("vt", dict(bufs=CFG.get("vt_bufs", 6))),
            ("sm", dict(bufs=4)),
            ("outp", dict(bufs=CFG["outp_bufs"])),
            ("ps_t", dict(bufs=CFG["ps_t"], space="PSUM")),  # [128,1024] bf16: 1 bank each
            ("ps_g", dict(bufs=1, space="PSUM")),
            ("ps_a", dict(bufs=CFG["ps_a"], space="PSUM")),  # [128,512] f32: 1 bank each
        ]:
            pools[name] = ctx.enter_context(tc.tile_pool(name=name, **kw))

        ident_f32 = pools["consts"].tile([128, 128], f32)
        masks.make_identity(nc, ident_f32)
        ident_bf16 = pools["consts"].tile([128, 128], mybir.dt.bfloat16)
        masks.make_identity(nc, ident_bf16)
        gamma_sb = pools["consts"].tile([128, 1], f32)
        nc.gpsimd.dma_start(out=gamma_sb, in_=g_d[:].to_broadcast((128, 1)))
        pools["ident_f32"] = ident_f32
        pools["ident_bf16"] = ident_bf16
        pools["gamma_sb"] = gamma_sb

        pools["_mode"] = mode
        if flat > 1:
            # loop-free repetition for TimelineSim period analysis
            carried = None
            if mode == "full":
                outd = ctx.enter_context(tc.tile_pool(name="outd", bufs=1))
                carried = []
                for p in range(4):
                    t = outd.tile([128, 2 * OUT_BLK], bf16, tag=f"od{p}")
                    nc.gpsimd.memset(t, 0)
                    carried.append(t)
            for _ in range(flat):
                _emit_workload(nc, mybir, pools, x_d, y_d, mode=mode,
                               carried=carried)
        elif reps:
            carried = None
            if mode == "full":
                # persistent sample-1 output tiles for the deferred-store
                # pipeline; zero-filled so iteration 0's deferred stores
                # read initialized data (overwritten by iterations >= 1)
                outd = ctx.enter_context(tc.tile_pool(name="outd", bufs=1))
                carried = []
                for p in range(4):
                    t = outd.tile([128, 2 * OUT_BLK], bf16, tag=f"od{p}")
                    nc.gpsimd.memset(t, 0)
                    carried.append(t)
            # PE body is several hundred instructions (> 1 IRAM block):
            # hint the back-edge prefetch so the bench loop doesn't pay an
            # I$ miss.
            with tc.For_i(0, reps, 1, hint_engines=(mybir.EngineType.PE,)):
                for _ in range(unroll):
                    _emit_workload(nc, mybir, pools, x_d, y_d, mode=mode,
                                   carried=carried)
        else:
            _emit_workload(nc, mybir, pools, x_d, y_d, mode=mode)

    nc.compile()
    return nc


_NC_CACHE = None


def _get_nc():
    global _NC_CACHE
    if _NC_CACHE is None:
        _NC_CACHE = _build_bass()
    return _NC_CACHE


def kernel(x, gamma, trace=False):
    from concourse.bass_utils import run_bass_kernel_spmd
    from concourse import mybir

    np_bf16 = mybir.dt.np(mybir.dt.bfloat16)
    x = np.asarray(x, dtype=np.float32).astype(np_bf16)
    gamma = np.asarray(gamma, dtype=np.float32)
    nc = _get_nc()

    xs = x.reshape(N_CORES, B_PER, C, HW)
    in_maps = [{"x": xs[i], "gamma": gamma} for i in range(N_CORES)]
    res = run_bass_kernel_spmd(nc, in_maps, core_ids=list(range(N_CORES)), trace=trace)
    out = np.stack([res.results[i]["y"] for i in range(N_CORES)], axis=0)
    out = out.astype(np.float32).reshape(B, C, H, W)
    if trace:
        return out, res
    return out

